# revision 1
# baseline (speedup 1.0000x reference)
"""GCNN (batched SpMM + GEMM + bias + ReLU) Trainium2 kernel.

Per-core work (one graph per NeuronCore, 8 graphs / 8 cores):
  phase 0: y = x @ W           (PE, fp32 in, bf16 out to DRAM)
  phase 1: out = relu(A @ y + b)
    - edges sorted by destination row (host-side index prep)
    - dma_gather y[cols] from DRAM (bf16, 256B rows)
    - one-hot segment matrices built on DVE (iota compare x vals)
    - segment-sum as PE matmuls accumulating into PSUM, 32-row windows
      col-tiled 4-per-PSUM-tile
    - bias + relu on eviction, DMA to DRAM

SPMD: one NEFF for all 8 cores. The chunk->window structure is baked
into the program, so per-window edge capacities are the max over all 8
graphs (rounded up to 128); each graph pads its windows with zero-val
edges.
"""

import sys

if "/opt/trn_rl_repo" not in sys.path:
    sys.path.insert(0, "/opt/trn_rl_repo")

import numpy as np
import ml_dtypes

import concourse.bacc as bacc
import concourse.mybir as mybir
from concourse import tile
from concourse.bass_utils import run_bass_kernel_spmd

BF16 = ml_dtypes.bfloat16

C = 128          # channels (C_IN == C_OUT == 128)
W_WIN = 32       # output rows per window (PSUM col-tile granularity)
WPG = 4          # windows per group (group = 128 output rows)
CALL_TARGET_CHUNKS = 56   # gather-call granularity (chunks)


# ---------------------------------------------------------------- host prep

def _round_up(a, m):
    return (a + m - 1) // m * m


def _prep(edge_rows, edge_cols, edge_vals, n_nodes):
    """Sort each graph's edges by destination row and pack them into a
    window structure shared by all graphs.

    Returns (structure, per_core_arrays).
    """
    Bn, En = edge_rows.shape
    n_win = _round_up(n_nodes, W_WIN) // W_WIN

    counts = np.zeros((Bn, n_win), dtype=np.int64)
    sorted_edges = []
    for g in range(Bn):
        order = np.argsort(edge_rows[g], kind="stable")
        rs = np.asarray(edge_rows[g])[order]
        cs = np.asarray(edge_cols[g])[order]
        vs = np.asarray(edge_vals[g])[order]
        wid = rs // W_WIN
        counts[g] = np.bincount(wid, minlength=n_win)
        sorted_edges.append((rs, cs, vs))

    cap = np.maximum(counts.max(axis=0), 1)
    cap = ((cap + 127) // 128 * 128).astype(np.int64)   # per-window capacity
    win_off = np.zeros(n_win + 1, dtype=np.int64)
    np.cumsum(cap, out=win_off[1:])
    total = int(win_off[-1])                             # padded edge count
    nch_w = (cap // 128).astype(np.int64)                # chunks per window
    chunk_window = np.repeat(np.arange(n_win), nch_w)    # chunk -> window id
    n_chunks = total // 128

    per_core = []
    for g in range(Bn):
        rs, cs, vs = sorted_edges[g]
        cols_p = np.zeros(total, dtype=np.int16)
        rloc_p = np.zeros(total, dtype=np.float32)
        vals_p = np.zeros(total, dtype=np.float32)
        src_off = np.zeros(n_win + 1, dtype=np.int64)
        np.cumsum(counts[g], out=src_off[1:])
        idx_dst = (win_off[:-1].repeat(counts[g])
                   + np.concatenate([np.arange(c) for c in counts[g]]))
        cols_p[idx_dst] = cs
        rloc_p[idx_dst] = rs - (rs // W_WIN) * W_WIN
        vals_p[idx_dst] = vs

        idx16 = np.tile(cols_p.reshape(-1, 16).T, (8, 1))          # [128, total/16]
        # host-built one-hot segment tiles: S[p, j, r] = vals[j*128+p]
        # iff rows_local[j*128+p] == r  (pure placement, no arithmetic)
        S = np.zeros((n_chunks, 128, W_WIN), dtype=BF16)
        jj = np.arange(total) // 128
        pp = np.arange(total) % 128
        S[jj, pp, rloc_p.astype(np.int64)] = vals_p.astype(BF16)
        S = np.ascontiguousarray(S.transpose(1, 0, 2))             # [128, n_chunks, 32]
        per_core.append((idx16, S))

    structure = (n_win, chunk_window, nch_w, total, n_chunks)
    return structure, per_core


def _make_calls(chunk_window, n_win):
    """Split the chunk list into dma_gather calls aligned to group
    boundaries: list of (chunk_lo, chunk_hi)."""
    n_chunks = len(chunk_window)
    group_of_chunk = chunk_window // WPG
    n_groups = int(group_of_chunk[-1]) + 1
    # chunk range per group
    grp_lo = np.searchsorted(group_of_chunk, np.arange(n_groups), side="left")
    grp_hi = np.searchsorted(group_of_chunk, np.arange(n_groups), side="right")
    calls = []
    lo = 0
    cur_lo_chunk = 0
    while lo < n_groups:
        hi = lo
        while hi < n_groups and (grp_hi[hi] - cur_lo_chunk) <= CALL_TARGET_CHUNKS:
            hi += 1
        if hi == lo:           # single huge group: take it anyway
            hi = lo + 1
        calls.append((int(grp_lo[lo]), int(grp_hi[hi - 1])))
        cur_lo_chunk = int(grp_hi[hi - 1])
        lo = hi
    assert calls[-1][1] == n_chunks
    return calls


# ---------------------------------------------------------------- device IR

def build_nc(n_nodes, structure, gather_dtype=mybir.dt.bfloat16):
    n_win, chunk_window, nch_w, total, n_chunks = structure
    n_tiles = _round_up(n_nodes, 128) // 128      # phase-0 row tiles
    n_groups = (n_win + WPG - 1) // WPG
    calls = _make_calls(chunk_window, n_win)
    max_call_chunks = max(hi - lo for lo, hi in calls)

    f32 = mybir.dt.float32
    bf16 = mybir.dt.bfloat16

    nc = bacc.Bacc("TRN2", num_swdge_queues=4)
    xT_d = nc.dram_tensor("xT", [C, n_nodes], f32, kind="ExternalInput")
    W_d = nc.dram_tensor("W", [C, C], f32, kind="ExternalInput")
    bb_d = nc.dram_tensor("b_bcast", [128, C], f32, kind="ExternalInput")
    idx_d = nc.dram_tensor("idx16", [128, total // 16], mybir.dt.int16,
                           kind="ExternalInput")
    s_d = nc.dram_tensor("S", [128, n_chunks * W_WIN], bf16, kind="ExternalInput")
    out_d = nc.dram_tensor("out", [n_nodes, C], f32, kind="ExternalOutput")
    y_d = nc.dram_tensor("y", [n_nodes, C], gather_dtype, kind="Internal")

    # chunk ranges per window for start/stop flags
    win_lo = {}
    win_hi = {}
    for j, w in enumerate(chunk_window):
        win_lo.setdefault(int(w), j)
        win_hi[int(w)] = j

    with tile.TileContext(nc) as tc:
        with (
            tc.tile_pool(name="const", bufs=1) as constp,
            tc.tile_pool(name="p0", bufs=3) as p0pool,
            tc.tile_pool(name="p0ps", bufs=2, space="PSUM") as p0ps,
            tc.tile_pool(name="gat", bufs=3) as gatp,
            tc.tile_pool(name="sm", bufs=3) as smp,
            tc.tile_pool(name="meta", bufs=2) as metap,
            tc.tile_pool(name="acc", bufs=4, space="PSUM") as accp,
            tc.tile_pool(name="ev", bufs=3) as evp,
        ):
            # ---- constants
            w_t = constp.tile([C, C], f32, tag="w")
            nc.sync.dma_start(out=w_t[:], in_=W_d[:])
            bias_t = constp.tile([128, C], f32, tag="bias")
            nc.sync.dma_start(out=bias_t[:], in_=bb_d[:])

            # ---- phase 0: y = x @ W  (tile over rows)
            for t in range(n_tiles):
                rows = min(128, n_nodes - t * 128)
                xt = p0pool.tile([C, 128], f32, tag="xt")
                nc.sync.dma_start(out=xt[:, :rows],
                                  in_=xT_d[:, t * 128:t * 128 + rows])
                yps = p0ps.tile([128, C], f32, tag="yps")
                nc.tensor.matmul(yps[:rows, :], xt[:, :rows], w_t[:],
                                 start=True, stop=True)
                ysb = p0pool.tile([128, C], gather_dtype, tag="ysb")
                nc.vector.tensor_copy(ysb[:rows, :], yps[:rows, :])
                nc.sync.dma_start(out=y_d[t * 128:t * 128 + rows, :],
                                  in_=ysb[:rows, :])

            tc.strict_bb_all_engine_barrier()

            # ---- phase 1: gather + segment matmul
            pending_psum = {}   # group id -> psum tile
            for ci, (c_lo, c_hi) in enumerate(calls):
                nch = c_hi - c_lo
                nidx = nch * 128
                idx_t = metap.tile([128, max_call_chunks * 8], mybir.dt.int16,
                                   tag="idx")
                nc.sync.dma_start(out=idx_t[:, :nch * 8],
                                  in_=idx_d[:, c_lo * 8:c_hi * 8])
                g_t = gatp.tile([128, max_call_chunks, C], gather_dtype, tag="g")
                nc.gpsimd.dma_gather(
                    out_ap=g_t[:, :nch, :],
                    in_ap=y_d[:],
                    idxs_ap=idx_t[:, :nch * 8],
                    num_idxs=nidx,
                    num_idxs_reg=nidx,
                    elem_size=C,
                    single_packet=False,
                    queue_num=ci % 4,
                )
                s_t = smp.tile([128, max_call_chunks, W_WIN], bf16, tag="s")
                nc.sync.dma_start(
                    out=s_t[:, :nch, :].rearrange("p a b -> p (a b)"),
                    in_=s_d[:, c_lo * W_WIN:c_hi * W_WIN])

                for j in range(c_lo, c_hi):
                    w = int(chunk_window[j])
                    grp, v = w // WPG, w % WPG
                    if grp not in pending_psum:
                        pending_psum[grp] = accp.tile([128, C], f32, tag="acc", name=f"acc_{grp}")
                    ps = pending_psum[grp]
                    nc.tensor.matmul(
                        ps[v * W_WIN:(v + 1) * W_WIN, :],
                        s_t[:, j - c_lo, :],
                        g_t[:, j - c_lo, :],
                        start=(j == win_lo[w]),
                        stop=(j == win_hi[w]),
                        tile_position=(0, v * W_WIN),
                        skip_group_check=True,
                    )
                    # group finished -> evict
                    last_win_of_grp = min((grp + 1) * WPG, n_win) - 1
                    if w == last_win_of_grp and j == win_hi[w]:
                        rows = min(128, n_nodes - grp * 128)
                        ot = evp.tile([128, C], f32, tag="ot")
                        nc.vector.tensor_tensor(
                            out=ot[:rows, :], in0=ps[:rows, :],
                            in1=bias_t[:rows, :], op=mybir.AluOpType.add)
                        nc.scalar.activation(
                            out=ot[:rows, :], in_=ot[:rows, :],
                            func=mybir.ActivationFunctionType.Relu)
                        nc.sync.dma_start(
                            out=out_d[grp * 128:grp * 128 + rows, :],
                            in_=ot[:rows, :])
                        del pending_psum[grp]

    nc.finalize()
    return nc


# ---------------------------------------------------------------- entry

def kernel(x, edge_rows, edge_cols, edge_vals, W, b):
    x = np.asarray(x, dtype=np.float32)
    edge_rows = np.asarray(edge_rows)
    edge_cols = np.asarray(edge_cols)
    edge_vals = np.asarray(edge_vals, dtype=np.float32)
    W = np.asarray(W, dtype=np.float32)
    b = np.asarray(b, dtype=np.float32)

    Bn, n_nodes, _ = x.shape
    structure, per_core = _prep(edge_rows, edge_cols, edge_vals, n_nodes)

    nc = build_nc(n_nodes, structure)

    bias_bcast = np.ascontiguousarray(
        np.broadcast_to(b.astype(np.float32), (128, C)))
    in_maps = []
    for g in range(Bn):
        idx16, S = per_core[g]
        in_maps.append({
            "xT": np.ascontiguousarray(x[g].T),
            "W": W,
            "b_bcast": bias_bcast,
            "idx16": idx16,
            "S": S.reshape(128, -1),
        })

    res = run_bass_kernel_spmd(nc, in_maps, list(range(Bn)))
    out = np.stack([np.asarray(r["out"], dtype=np.float32) for r in res.results])
    return out



# revision 2
# speedup vs baseline: 3.0894x; 3.0894x over previous
"""GCNN (batched SpMM + GEMM + bias + ReLU) Trainium2 kernel.

Strategy: dense block-streamed SpMM (no gather, no per-edge DMA
descriptors).

Per-core work (one graph per NeuronCore, 8 graphs / 8 cores):
  phase 0: y = x @ W            (bf16 PE matmuls, y tiles stay in SBUF)
  phase 1: out^T = A^T-blocks streamed dense:
      out^T[ch, dest] = sum_k y_k^T @ A^T[k-block, dest-block]
    - A^T built dense on host as [10112, 10112] bf16 (79x79 blocks of
      128x128); zero-padded; streamed HBM->SBUF in ~768 KB slabs at
      full bandwidth as the matmul *moving* operand
    - y_k (bf16, SBUF-resident) is the *stationary* operand
    - PSUM accumulates out^T per dest-group over all 79 k-blocks;
      4 dest passes of <=24 groups (6 PSUM banks, 4 groups per bank)
    - bias + relu on eviction (bias is per-partition in the out^T
      orientation), DMA out^T to DRAM; host transposes back

The SWDGE dma_gather approach (one descriptor per edge) is descriptor-
generation-bound on the GPSIMD engine (~6 ns/edge = 2.2 ms/core); the
dense stream moves more bytes (~205 MB vs ~115 MB) but at full DMA
bandwidth with zero GPSIMD work and pure PE streaming.

SPMD: one NEFF for all 8 cores; per-core tensors differ only in data.
"""

import sys

if "/opt/trn_rl_repo" not in sys.path:
    sys.path.insert(0, "/opt/trn_rl_repo")

import numpy as np
import ml_dtypes

import concourse.bacc as bacc
import concourse.mybir as mybir
from concourse import tile
from concourse.bass_utils import run_bass_kernel_spmd

BF16 = ml_dtypes.bfloat16

C = 128            # channels (C_IN == C_OUT == 128)
N = 10000          # nodes per graph
NB = (N + 127) // 128          # 79 node blocks
NPAD = NB * 128                # 10112
GROUPS_PER_PASS = 24           # 6 PSUM banks x 4 groups per bank
BANK_COLS = 512                # fp32 columns per PSUM bank


def _passes():
    out = []
    g = 0
    while g < NB:
        out.append((g, min(g + GROUPS_PER_PASS, NB)))
        g += GROUPS_PER_PASS
    return out


# ---------------------------------------------------------------- host prep

def prep_in_maps(x, edge_rows, edge_cols, edge_vals, W, b):
    """Build per-core input maps: xT (bf16), W (bf16), bT (f32 bias
    broadcast in out^T orientation), AT (dense A^T, bf16, padded)."""
    x = np.asarray(x)
    W16 = np.asarray(W, dtype=np.float32).astype(BF16)
    bT = np.ascontiguousarray(
        np.broadcast_to(np.asarray(b, dtype=np.float32)[:, None], (C, BANK_COLS)))
    in_maps = []
    for g in range(x.shape[0]):
        at = np.zeros(NPAD * NPAD, dtype=np.float32)
        idx = (np.asarray(edge_cols[g], dtype=np.int64) * NPAD
               + np.asarray(edge_rows[g], dtype=np.int64))
        np.add.at(at, idx, np.asarray(edge_vals[g], dtype=np.float32))
        in_maps.append({
            "xT": np.ascontiguousarray(x[g].T.astype(BF16)),
            "W": W16,
            "bT": bT,
            "AT": at.reshape(NPAD, NPAD).astype(BF16),
        })
    return in_maps


# ---------------------------------------------------------------- device IR

def build_nc():
    f32 = mybir.dt.float32
    bf16 = mybir.dt.bfloat16

    nc = bacc.Bacc("TRN2")
    xT_d = nc.dram_tensor("xT", [C, N], bf16, kind="ExternalInput")
    W_d = nc.dram_tensor("W", [C, C], bf16, kind="ExternalInput")
    bT_d = nc.dram_tensor("bT", [C, BANK_COLS], f32, kind="ExternalInput")
    AT_d = nc.dram_tensor("AT", [NPAD, NPAD], bf16, kind="ExternalInput")
    outT_d = nc.dram_tensor("outT", [C, N], f32, kind="ExternalOutput")

    passes = _passes()
    max_span = GROUPS_PER_PASS * 128

    with tile.TileContext(nc) as tc:
        with (
            tc.tile_pool(name="const", bufs=1) as constp,
            tc.tile_pool(name="ypool", bufs=NB) as ypool,
            tc.tile_pool(name="p0ps", bufs=2, space="PSUM") as p0ps,
            tc.tile_pool(name="atp", bufs=3) as atp,
            tc.tile_pool(name="acc", bufs=6, space="PSUM") as accp,
            tc.tile_pool(name="ev", bufs=3) as evp,
        ):
            # ---- constants
            w_t = constp.tile([C, C], bf16, tag="w")
            nc.sync.dma_start(out=w_t[:], in_=W_d[:])
            bias_t = constp.tile([C, BANK_COLS], f32, tag="bias")
            nc.sync.dma_start(out=bias_t[:], in_=bT_d[:])
            x_t = constp.tile([C, N], bf16, tag="x")
            nc.sync.dma_start(out=x_t[:], in_=xT_d[:])

            # ---- phase 0: y = x @ W, tiles kept resident in SBUF (bf16)
            y_tiles = []
            for t in range(NB):
                rows = min(128, N - t * 128)
                yps = p0ps.tile([128, C], f32, tag="yps")
                nc.tensor.matmul(yps[:rows, :],
                                 x_t[:, t * 128:t * 128 + rows],
                                 w_t[:], start=True, stop=True)
                ysb = ypool.tile([128, C], bf16, tag="y", name=f"y_{t}")
                if rows < 128:
                    nc.vector.memset(ysb[:], 0.0)
                nc.vector.tensor_copy(ysb[:rows, :], yps[:rows, :])
                y_tiles.append(ysb)

            # ---- phase 1: stream A^T blocks, accumulate out^T in PSUM
            for (g0, g1) in passes:
                span = (g1 - g0) * 128
                nbank = (span + BANK_COLS - 1) // BANK_COLS
                ps = []
                for bi in range(nbank):
                    pt = accp.tile([128, BANK_COLS], f32, tag="acc",
                                   name=f"acc_{g0}_{bi}")
                    ps.append(pt)
                for k in range(NB):
                    at_t = atp.tile([128, max_span], bf16, tag="at")
                    nc.sync.dma_start(
                        out=at_t[:, :span],
                        in_=AT_d[k * 128:(k + 1) * 128,
                                 g0 * 128:g0 * 128 + span])
                    for bi in range(nbank):
                        ncols = min(BANK_COLS, span - bi * BANK_COLS)
                        nc.tensor.matmul(
                            ps[bi][:, :ncols],
                            y_tiles[k][:],
                            at_t[:, bi * BANK_COLS:bi * BANK_COLS + ncols],
                            start=(k == 0), stop=(k == NB - 1),
                            skip_group_check=True,
                        )
                # evict: bias + relu + DMA out (clamped to real cols)
                for bi in range(nbank):
                    col0 = g0 * 128 + bi * BANK_COLS
                    real = min(BANK_COLS, N - col0)
                    if real <= 0:
                        continue
                    ot = evp.tile([128, BANK_COLS], f32, tag="ot")
                    nc.vector.tensor_tensor(
                        out=ot[:, :real], in0=ps[bi][:, :real],
                        in1=bias_t[:, :real], op=mybir.AluOpType.add)
                    nc.scalar.activation(
                        out=ot[:, :real], in_=ot[:, :real],
                        func=mybir.ActivationFunctionType.Relu)
                    nc.sync.dma_start(out=outT_d[:, col0:col0 + real],
                                      in_=ot[:, :real])

    nc.finalize()
    return nc


# ---------------------------------------------------------------- entry

def kernel(x, edge_rows, edge_cols, edge_vals, W, b):
    x = np.asarray(x)
    in_maps = prep_in_maps(x, edge_rows, edge_cols, edge_vals, W, b)
    nc = build_nc()
    res = run_bass_kernel_spmd(nc, in_maps, list(range(x.shape[0])))
    out = np.stack([np.asarray(r["outT"], dtype=np.float32).T
                    for r in res.results])
    return out


# revision 3
# speedup vs baseline: 3.7199x; 1.2041x over previous
"""GCNN (batched SpMM + GEMM + bias + ReLU) Trainium2 kernel.

Strategy: dense block-streamed SpMM (no gather, no per-edge DMA
descriptors).

Per-core work (one graph per NeuronCore, 8 graphs / 8 cores):
  phase 0: y = x @ W            (bf16 PE matmuls, y tiles stay in SBUF)
  phase 1: out^T = A^T-blocks streamed dense:
      out^T[ch, dest] = sum_k y_k^T @ A^T[k-block, dest-block]
    - A^T built dense on host, bf16, in a pass-major layout
      [128 src-lane, pass | k | group | dest] so each DMA call moves
      KCHUNK k-blocks with one large contiguous descriptor per
      partition (~48 KB) at full HBM bandwidth
    - y_k (bf16, SBUF-resident) is the matmul *stationary* operand;
      A^T slabs are the *moving* operand
    - PSUM accumulates out^T per dest-group over all 79 k-blocks;
      4 dest passes of <=24 groups (6 PSUM banks, 4 groups per bank)
    - bias + relu on eviction (bias is per-partition in the out^T
      orientation), DMA out^T to DRAM; host transposes back

The SWDGE dma_gather approach (one descriptor per edge) is descriptor-
generation-bound on the GPSIMD engine (~6 ns/edge = 2.2 ms/core); the
dense stream moves more bytes (~205 MB vs ~115 MB) but at full DMA
bandwidth with zero GPSIMD work and pure PE streaming.

SPMD: one NEFF for all 8 cores; per-core tensors differ only in data.
"""

import sys

if "/opt/trn_rl_repo" not in sys.path:
    sys.path.insert(0, "/opt/trn_rl_repo")

import numpy as np
import ml_dtypes

import concourse.bacc as bacc
import concourse.mybir as mybir
from concourse import tile
from concourse.bass_utils import run_bass_kernel_spmd

BF16 = ml_dtypes.bfloat16

C = 128            # channels (C_IN == C_OUT == 128)
N = 10000          # nodes per graph
NB = (N + 127) // 128          # 79 node blocks
NPAD = NB * 128                # 10112
GROUPS_PER_PASS = 24           # 6 PSUM banks x 4 groups per bank
BANK_COLS = 512                # fp32 columns per PSUM bank
KCHUNK = 8                     # k-blocks per DMA slab


def _passes():
    out = []
    g = 0
    while g < NB:
        out.append((g, min(g + GROUPS_PER_PASS, NB)))
        g += GROUPS_PER_PASS
    return out


PASSES = _passes()
PASS_SPAN = [(g1 - g0) * 128 for g0, g1 in PASSES]
PASS_OFF = np.concatenate([[0], np.cumsum([NB * s for s in PASS_SPAN])])
AT_COLS = int(PASS_OFF[-1])    # 79 * 10112 = 798848


# ---------------------------------------------------------------- host prep

def prep_in_maps(x, edge_rows, edge_cols, edge_vals, W, b):
    """Build per-core input maps: xT (bf16), W (bf16), bT (f32 bias
    broadcast in out^T orientation), AT (dense A^T, bf16, pass-major)."""
    x = np.asarray(x)
    W16 = np.asarray(W, dtype=np.float32).astype(BF16)
    bT = np.ascontiguousarray(
        np.broadcast_to(np.asarray(b, dtype=np.float32)[:, None], (C, BANK_COLS)))

    # column index within the pass-major AT for each edge
    def at_flat_index(rows, cols):
        k = cols // 128
        c_loc = cols % 128
        g = rows // 128
        r_loc = rows % 128
        p = g // GROUPS_PER_PASS
        g_loc = g % GROUPS_PER_PASS
        span = np.array(PASS_SPAN, dtype=np.int64)[p]
        col = PASS_OFF[p] + k * span + g_loc * 128 + r_loc
        return c_loc * AT_COLS + col

    in_maps = []
    for g in range(x.shape[0]):
        at = np.zeros(C * AT_COLS, dtype=np.float32)
        idx = at_flat_index(np.asarray(edge_rows[g], dtype=np.int64),
                            np.asarray(edge_cols[g], dtype=np.int64))
        np.add.at(at, idx, np.asarray(edge_vals[g], dtype=np.float32))
        in_maps.append({
            "xT": np.ascontiguousarray(x[g].T.astype(BF16)),
            "W": W16,
            "bT": bT,
            "AT": at.reshape(C, AT_COLS).astype(BF16),
        })
    return in_maps


# ---------------------------------------------------------------- device IR

def build_nc():
    f32 = mybir.dt.float32
    bf16 = mybir.dt.bfloat16

    nc = bacc.Bacc("TRN2")
    xT_d = nc.dram_tensor("xT", [C, N], bf16, kind="ExternalInput")
    W_d = nc.dram_tensor("W", [C, C], bf16, kind="ExternalInput")
    bT_d = nc.dram_tensor("bT", [C, BANK_COLS], f32, kind="ExternalInput")
    AT_d = nc.dram_tensor("AT", [C, AT_COLS], bf16, kind="ExternalInput")
    outT_d = nc.dram_tensor("outT", [C, N], f32, kind="ExternalOutput")

    max_slab = KCHUNK * GROUPS_PER_PASS * 128   # bf16 elems per partition

    with tile.TileContext(nc) as tc:
        with (
            tc.tile_pool(name="const", bufs=1) as constp,
            tc.tile_pool(name="ypool", bufs=NB) as ypool,
            tc.tile_pool(name="p0ps", bufs=2, space="PSUM") as p0ps,
            tc.tile_pool(name="atp", bufs=2) as atp,
            tc.tile_pool(name="acc", bufs=6, space="PSUM") as accp,
            tc.tile_pool(name="ev", bufs=3) as evp,
        ):
            # ---- constants
            w_t = constp.tile([C, C], bf16, tag="w")
            nc.sync.dma_start(out=w_t[:], in_=W_d[:])
            bias_t = constp.tile([C, BANK_COLS], f32, tag="bias")
            nc.sync.dma_start(out=bias_t[:], in_=bT_d[:])
            x_t = constp.tile([C, N], bf16, tag="x")
            nc.sync.dma_start(out=x_t[:], in_=xT_d[:])

            # ---- phase 0: y = x @ W, tiles kept resident in SBUF (bf16)
            y_tiles = []
            for t in range(NB):
                rows = min(128, N - t * 128)
                yps = p0ps.tile([128, C], f32, tag="yps")
                nc.tensor.matmul(yps[:rows, :],
                                 x_t[:, t * 128:t * 128 + rows],
                                 w_t[:], start=True, stop=True)
                ysb = ypool.tile([128, C], bf16, tag="y", name=f"y_{t}")
                if rows < 128:
                    nc.vector.memset(ysb[:], 0.0)
                nc.vector.tensor_copy(ysb[:rows, :], yps[:rows, :])
                y_tiles.append(ysb)

            # ---- phase 1: stream A^T slabs, accumulate out^T in PSUM
            dma_engines = [nc.sync, nc.scalar]
            slab_i = 0
            for pi, (g0, g1) in enumerate(PASSES):
                span = PASS_SPAN[pi]
                nbank = (span + BANK_COLS - 1) // BANK_COLS
                ps = []
                for bi in range(nbank):
                    pt = accp.tile([128, BANK_COLS], f32, tag="acc",
                                   name=f"acc_{g0}_{bi}")
                    ps.append(pt)
                for k0 in range(0, NB, KCHUNK):
                    kn = min(KCHUNK, NB - k0)
                    at_t = atp.tile([128, max_slab], bf16, tag="at")
                    lo = int(PASS_OFF[pi]) + k0 * span
                    dma_engines[slab_i % 2].dma_start(
                        out=at_t[:, :kn * span],
                        in_=AT_d[:, lo:lo + kn * span])
                    slab_i += 1
                    for kk in range(kn):
                        k = k0 + kk
                        for bi in range(nbank):
                            ncols = min(BANK_COLS, span - bi * BANK_COLS)
                            off = kk * span + bi * BANK_COLS
                            nc.tensor.matmul(
                                ps[bi][:, :ncols],
                                y_tiles[k][:],
                                at_t[:, off:off + ncols],
                                start=(k == 0), stop=(k == NB - 1),
                                skip_group_check=True,
                            )
                # evict: bias + relu + DMA out (clamped to real cols)
                for bi in range(nbank):
                    col0 = g0 * 128 + bi * BANK_COLS
                    real = min(BANK_COLS, N - col0)
                    if real <= 0:
                        continue
                    ot = evp.tile([128, BANK_COLS], f32, tag="ot")
                    nc.vector.tensor_tensor(
                        out=ot[:, :real], in0=ps[bi][:, :real],
                        in1=bias_t[:, :real], op=mybir.AluOpType.add)
                    nc.scalar.activation(
                        out=ot[:, :real], in_=ot[:, :real],
                        func=mybir.ActivationFunctionType.Relu)
                    nc.sync.dma_start(out=outT_d[:, col0:col0 + real],
                                      in_=ot[:, :real])

    nc.finalize()
    return nc


# ---------------------------------------------------------------- entry

def kernel(x, edge_rows, edge_cols, edge_vals, W, b):
    x = np.asarray(x)
    in_maps = prep_in_maps(x, edge_rows, edge_cols, edge_vals, W, b)
    nc = build_nc()
    res = run_bass_kernel_spmd(nc, in_maps, list(range(x.shape[0])))
    out = np.stack([np.asarray(r["outT"], dtype=np.float32).T
                    for r in res.results])
    return out


# revision 4
# speedup vs baseline: 3.8091x; 1.0240x over previous
"""GCNN (batched SpMM + GEMM + bias + ReLU) Trainium2 kernel.

Strategy: dense block-streamed SpMM (no gather, no per-edge DMA
descriptors).

Per-core work (one graph per NeuronCore, 8 graphs / 8 cores):
  phase 0: y = x @ W            (bf16 PE matmuls, y tiles stay in SBUF)
  phase 1: out^T = A^T-blocks streamed dense:
      out^T[ch, dest] = sum_k y_k^T @ A^T[k-block, dest-block]
    - A^T built dense on host, bf16, in a pass-major layout
      [128 src-lane, pass | k | dest-col] so each DMA call moves
      KCHUNK k-blocks with one large contiguous descriptor per
      partition (~24 KB) at full HBM bandwidth
    - y_k (bf16, SBUF-resident) is the matmul *stationary* operand;
      A^T slabs are the *moving* operand
    - PSUM accumulates out^T per dest pass over all 79 k-blocks;
      7 dest passes of 12 groups (3 PSUM banks each) ping-pong across
      6 PSUM banks so evictions overlap the next pass's matmuls
    - eviction: single ACT op relu(psum + bias) (bias is per-partition
      in the out^T orientation), DMA out^T to DRAM; host transposes

The SWDGE dma_gather approach (one descriptor per edge) is descriptor-
generation-bound on the GPSIMD engine (~6 ns/edge = 2.2 ms/core); the
dense stream moves more bytes (~203 MB vs ~115 MB) but at full DMA
bandwidth with zero GPSIMD work and pure PE streaming.

SPMD: one NEFF for all 8 cores; per-core tensors differ only in data.
"""

import sys

if "/opt/trn_rl_repo" not in sys.path:
    sys.path.insert(0, "/opt/trn_rl_repo")

import numpy as np
import ml_dtypes

import concourse.bacc as bacc
import concourse.mybir as mybir
from concourse import tile
from concourse.bass_utils import run_bass_kernel_spmd

BF16 = ml_dtypes.bfloat16

C = 128            # channels (C_IN == C_OUT == 128)
N = 10000          # nodes per graph
NB = (N + 127) // 128          # 79 node blocks
GROUPS_PER_PASS = 12           # 3 PSUM banks per pass, 6 banks ping-pong
BANK_COLS = 512                # fp32 columns per PSUM bank
KCHUNK = 8                     # k-blocks per DMA slab


def _passes():
    out = []
    g = 0
    while g * 128 < N:
        c0 = g * 128
        c1 = min((g + GROUPS_PER_PASS) * 128, N)
        out.append((c0, c1 - c0))          # (first dest col, real span)
        g += GROUPS_PER_PASS
    return out


PASSES = _passes()                          # [(col0, span), ...]
PASS_OFF = np.concatenate(
    [[0], np.cumsum([NB * s for _, s in PASSES])]).astype(np.int64)
AT_COLS = int(PASS_OFF[-1])                 # 79 * 10000 = 790000


# ---------------------------------------------------------------- host prep

def prep_in_maps(x, edge_rows, edge_cols, edge_vals, W, b):
    """Build per-core input maps: xT (bf16), W (bf16), bT (f32 bias in
    out^T orientation), AT (dense A^T, bf16, pass-major layout)."""
    x = np.asarray(x)
    W16 = np.asarray(W, dtype=np.float32).astype(BF16)
    bT = np.ascontiguousarray(
        np.asarray(b, dtype=np.float32)[:, None] * np.ones((1, 1), np.float32))

    spans = np.array([s for _, s in PASSES], dtype=np.int64)

    def at_flat_index(rows, cols):
        k = cols // 128
        c_loc = cols % 128
        p = rows // (GROUPS_PER_PASS * 128)
        col = PASS_OFF[p] + k * spans[p] + (rows - p * GROUPS_PER_PASS * 128)
        return c_loc * AT_COLS + col

    in_maps = []
    for g in range(x.shape[0]):
        at = np.zeros(C * AT_COLS, dtype=np.float32)
        idx = at_flat_index(np.asarray(edge_rows[g], dtype=np.int64),
                            np.asarray(edge_cols[g], dtype=np.int64))
        np.add.at(at, idx, np.asarray(edge_vals[g], dtype=np.float32))
        in_maps.append({
            "xT": np.ascontiguousarray(x[g].T.astype(BF16)),
            "W": W16,
            "bT": bT,
            "AT": at.reshape(C, AT_COLS).astype(BF16),
        })
    return in_maps


# ---------------------------------------------------------------- device IR

def build_nc():
    f32 = mybir.dt.float32
    bf16 = mybir.dt.bfloat16

    nc = bacc.Bacc("TRN2")
    xT_d = nc.dram_tensor("xT", [C, N], bf16, kind="ExternalInput")
    W_d = nc.dram_tensor("W", [C, C], bf16, kind="ExternalInput")
    bT_d = nc.dram_tensor("bT", [C, 1], f32, kind="ExternalInput")
    AT_d = nc.dram_tensor("AT", [C, AT_COLS], bf16, kind="ExternalInput")
    outT_d = nc.dram_tensor("outT", [C, N], f32, kind="ExternalOutput")

    max_slab = KCHUNK * GROUPS_PER_PASS * 128   # bf16 elems per partition

    with tile.TileContext(nc) as tc:
        with (
            tc.tile_pool(name="const", bufs=1) as constp,
            tc.tile_pool(name="ypool", bufs=NB) as ypool,
            tc.tile_pool(name="p0ps", bufs=2, space="PSUM") as p0ps,
            tc.tile_pool(name="atp", bufs=4) as atp,
            tc.tile_pool(name="acc", bufs=6, space="PSUM") as accp,
            tc.tile_pool(name="ev", bufs=3) as evp,
        ):
            # ---- constants
            w_t = constp.tile([C, C], bf16, tag="w")
            nc.sync.dma_start(out=w_t[:], in_=W_d[:])
            bias_t = constp.tile([C, 1], f32, tag="bias")
            nc.sync.dma_start(out=bias_t[:], in_=bT_d[:])
            x_t = constp.tile([C, N], bf16, tag="x")
            nc.sync.dma_start(out=x_t[:], in_=xT_d[:])

            # ---- phase 0: y = x @ W, tiles kept resident in SBUF (bf16)
            y_tiles = []
            for t in range(NB):
                rows = min(128, N - t * 128)
                yps = p0ps.tile([128, C], f32, tag="yps")
                nc.tensor.matmul(yps[:rows, :],
                                 x_t[:, t * 128:t * 128 + rows],
                                 w_t[:], start=True, stop=True)
                ysb = ypool.tile([128, C], bf16, tag="y", name=f"y_{t}")
                if rows < 128:
                    nc.vector.memset(ysb[:], 0.0)
                nc.vector.tensor_copy(ysb[:rows, :], yps[:rows, :])
                y_tiles.append(ysb)

            # ---- phase 1: stream A^T slabs, accumulate out^T in PSUM
            dma_engines = [nc.sync, nc.scalar]
            slab_i = 0
            for pi, (col0, span) in enumerate(PASSES):
                nbank = (span + BANK_COLS - 1) // BANK_COLS
                ps = []
                for bi in range(nbank):
                    pt = accp.tile([128, BANK_COLS], f32, tag="acc",
                                   name=f"acc_{col0}_{bi}")
                    ps.append(pt)
                for k0 in range(0, NB, KCHUNK):
                    kn = min(KCHUNK, NB - k0)
                    at_t = atp.tile([128, max_slab], bf16, tag="at")
                    lo = int(PASS_OFF[pi]) + k0 * span
                    dma_engines[slab_i % 2].dma_start(
                        out=at_t[:, :kn * span],
                        in_=AT_d[:, lo:lo + kn * span])
                    slab_i += 1
                    for kk in range(kn):
                        k = k0 + kk
                        for bi in range(nbank):
                            ncols = min(BANK_COLS, span - bi * BANK_COLS)
                            off = kk * span + bi * BANK_COLS
                            nc.tensor.matmul(
                                ps[bi][:, :ncols],
                                y_tiles[k][:],
                                at_t[:, off:off + ncols],
                                start=(k == 0), stop=(k == NB - 1),
                                skip_group_check=True,
                            )
                # evict: relu(psum + bias) in one ACT op, then DMA out
                for bi in range(nbank):
                    c0 = col0 + bi * BANK_COLS
                    real = min(BANK_COLS, col0 + span - c0)
                    ot = evp.tile([128, BANK_COLS], f32, tag="ot")
                    nc.scalar.activation(
                        out=ot[:, :real], in_=ps[bi][:, :real],
                        func=mybir.ActivationFunctionType.Relu,
                        bias=bias_t[:, 0:1])
                    nc.sync.dma_start(out=outT_d[:, c0:c0 + real],
                                      in_=ot[:, :real])

    nc.finalize()
    return nc


# ---------------------------------------------------------------- entry

def kernel(x, edge_rows, edge_cols, edge_vals, W, b):
    x = np.asarray(x)
    in_maps = prep_in_maps(x, edge_rows, edge_cols, edge_vals, W, b)
    nc = build_nc()
    res = run_bass_kernel_spmd(nc, in_maps, list(range(x.shape[0])))
    out = np.stack([np.asarray(r["outT"], dtype=np.float32).T
                    for r in res.results])
    return out
